# revision 5
# baseline (speedup 1.0000x reference)
"""Bidirectional 2-layer LSTM encoder on 8 Trainium2 NeuronCores.

Problem shapes (hardcoded): B=64, T=512, IN=H=512, L=2.

Sharding: 4 worker roles = (direction in {fwd,bwd}) x (batch half of 32).
Each role runs BOTH layers of its direction on its 32 batch rows, with
layer 1 lagging layer 0 by LAG steps (same-core pipeline). Cores 4-7
duplicate cores 0-3 (outputs ignored) so all 8 cores run one SPMD program.

Per-core per-step structure (per layer):
  PSUM gates G [4*32, 512] = bias (K=1 ones-MM) + x-part + recurrent h-part,
  all float32r col-tiled matmuls (gate g -> PE col-group g -> partitions 32g).
  Gate order remapped to (i, f, o, g) so one sigmoid covers partitions 0-95.
  c update and h = sig(o)*tanh(c) on DVE/GPSIMD; h transposed back to
  hidden-major via 4 PE transposes to become the next step's stationary.
"""

import numpy as np
from contextlib import ExitStack

B, T, IN, H, L = 64, 512, 512, 512, 2
NB = 32          # batch rows per core (B/2 halves)
CH = 16          # x-chunk steps resident in SBUF
LAG = 8          # layer-1 lag in steps
N_CORES = 8

_compiled = None
ABLATE = set()


def _build_program():
    import concourse.bass as bass
    import concourse.tile as tile
    from concourse import bacc, mybir
    from concourse.masks import make_identity

    F32 = mybir.dt.float32
    F16 = mybir.dt.float16
    AF = mybir.ActivationFunctionType
    MUL = mybir.AluOpType.mult
    ADD = mybir.AluOpType.add

    nc = bacc.Bacc("TRN2", target_bir_lowering=False, debug=False)

    # ---- DRAM parameters ----
    xT_d = nc.declare_dram_parameter("xT", [128, T, 128], F16, isOutput=False)
    w_d = {}
    for l in range(L):
        w_d[("Wh", l)] = nc.declare_dram_parameter(f"WhT{l}", [128, 4 * 4 * H], F16, isOutput=False)
        w_d[("Wi", l)] = nc.declare_dram_parameter(f"WiT{l}", [128, 4 * 4 * H], F16, isOutput=False)
        w_d[("b", l)] = nc.declare_dram_parameter(f"bias{l}", [1, 4 * H], F16, isOutput=False)
    outs_d = nc.declare_dram_parameter("outs", [T, NB, H], F32, isOutput=True)
    hfin_d = nc.declare_dram_parameter("hfin", [L, NB, H], F32, isOutput=True)

    with tile.TileContext(nc) as tc, ExitStack() as ctx:
        wp = ctx.enter_context(tc.tile_pool(name="wp", bufs=1))
        xp = ctx.enter_context(tc.tile_pool(name="xp", bufs=2))
        hr = ctx.enter_context(tc.tile_pool(name="hr", bufs=LAG + 4))
        h1r = ctx.enter_context(tc.tile_pool(name="h1r", bufs=3))
        ewp = ctx.enter_context(tc.tile_pool(name="ewp", bufs=3))
        cp = ctx.enter_context(tc.tile_pool(name="cp", bufs=1))
        psG = ctx.enter_context(tc.tile_pool(name="psG", bufs=4, space="PSUM"))
        psT = ctx.enter_context(tc.tile_pool(name="psT", bufs=2, space="PSUM"))
        psP = ctx.enter_context(tc.tile_pool(name="psP", bufs=2, space="PSUM"))

        # ---- resident weights ----
        W = {}
        for key in [("Wh", 0), ("Wi", 0), ("Wh", 1), ("Wi", 1)]:
            t = wp.tile([128, 4 * 4 * H], F16, tag=f"w{key[0]}{key[1]}", name=f"w{key[0]}{key[1]}")
            nc.sync.dma_start(t[:], w_d[key][:])
            W[key] = t
        bias = {}
        for l in range(L):
            t = wp.tile([1, 4 * H], F16, tag=f"b{l}", name=f"bias_t{l}")
            nc.sync.dma_start(t[:], w_d[("b", l)][:])
            bias[l] = t
        ones = wp.tile([1, NB], F16, tag="ones")
        nc.vector.memset(ones[:], 1.0)
        ident = wp.tile([128, NB], F32, tag="ident")
        make_identity(nc, ident[64:96, :])

        # ---- persistent state ----
        # c lives at partitions 32:64 of a [64, 512] tile (aligned with f')
        c_t = {l: cp.tile([64, H], F32, tag=f"c{l}", name=f"c{l}") for l in range(L)}
        for l in range(L):
            nc.vector.memset(c_t[l][:], 0.0)
        hT0 = {}
        for l in range(L):
            t = wp.tile([128, 128], F16, tag=f"hT0_{l}", name=f"hT0_{l}")
            nc.vector.memset(t[:], 0.0)
            hT0[l] = t

        def lstm_step(l, t_idx, hT_prev, x_stat, x_off, is_last):
            """One LSTM step for layer l at time t_idx.
            hT_prev: [128,128] f32r tile, hidden-major h_{t-1} (4 k-tiles of 32 cols)
            x_stat:  [128, *] f32r tile holding input^T; k-tile kk at
                     x_off + kk*32 .. +32 columns.
            Returns (hT_new, Hout tile)."""
            G = psG.tile([128, 4 * H // 4], F32, tag="G")   # [128, 512]
            Wi_t, Wh_t, b_t = W[("Wi", l)], W[("Wh", l)], bias[l]
            for g in range(4):
                gs = G[32 * g:32 * g + 32, :]
                tp = (0, 32 * g)
                nc.tensor.matmul(gs, ones[:], b_t[:, 512 * g:512 * g + 512],
                                 start=True, stop=("no_x" in ABLATE and "no_h" in ABLATE), tile_position=tp)
                if "no_x" not in ABLATE:
                    for kk in range(4):
                        nc.tensor.matmul(
                            gs, x_stat[:, x_off + 32 * kk:x_off + 32 * kk + 32],
                            Wi_t[:, 2048 * kk + 512 * g:2048 * kk + 512 * g + 512],
                            start=False, stop=("no_h" in ABLATE and kk == 3), tile_position=tp)
                if "no_h" not in ABLATE:
                    for kk in range(4):
                        nc.tensor.matmul(
                            gs, hT_prev[:, 32 * kk:32 * kk + 32],
                            Wh_t[:, 2048 * kk + 512 * g:2048 * kk + 512 * g + 512],
                            start=False, stop=(kk == 3), tile_position=tp)

            S = ewp.tile([128, H], F32, tag="S")     # i'@0, f'@32, o'@64
            EW = ewp.tile([128, H], F32, tag="EW")   # tanh_g@0, t2@32, th@64
            HO = ewp.tile([128, H], F32, tag="HO")   # h@64
            nc.scalar.activation(S[0:96, :], G[0:96, :], AF.Sigmoid)
            nc.scalar.activation(EW[0:32, :], G[96:128, :], AF.Tanh)
            P1 = psP.tile([32, H], F32, tag="P1")
            # t1 = sig(i) * tanh(g)   (DVE; SBUF ins @0, out PSUM)
            nc.vector.tensor_tensor(P1[:], EW[0:32, :], S[0:32, :], MUL)
            # t2 = sig(f) * c        (GPSIMD; all SBUF @32)
            nc.gpsimd.tensor_tensor(EW[32:64, :], S[32:64, :], c_t[l][32:64, :], MUL)
            # c = t1 + t2            (DVE; PSUM in0 reloc, SBUF @32)
            nc.vector.tensor_tensor(c_t[l][32:64, :], P1[:], EW[32:64, :], ADD)
            # th = tanh(c)           (ACT; reloc 32->64)
            nc.scalar.activation(EW[64:96, :], c_t[l][32:64, :], AF.Tanh)
            # h = sig(o) * th        (DVE; all SBUF @64)
            nc.vector.tensor_tensor(HO[64:96, :], S[64:96, :], EW[64:96, :], MUL)

            # transpose h -> hidden-major stationary for next step
            pool = h1r if l == 1 else hr
            hT_new = pool.tile([128, 128], F16, tag=f"hT{l}", name=f"hT{l}")
            if "no_tr" in ABLATE:
                nc.vector.memset(hT_new[:], 0.0)
            else:
                trP = psT.tile([128, 128], F32, tag="trP")
                for kk in range(4):
                    nc.tensor.transpose(trP[:, 32 * kk:32 * kk + 32],
                                        HO[64:96, 128 * kk:128 * kk + 128],
                                        ident[64:96, :])
                nc.scalar.copy(hT_new[:], trP[:])

            if l == 1:
                nc.sync.dma_start(outs_d[t_idx], HO[64:96, :])
            if is_last:
                nc.sync.dma_start(hfin_d[l], HO[64:96, :])
            return hT_new

        # ---- main loop ----
        hT_l0 = hT0[0]
        hT_l1 = hT0[1]
        ring = {}          # t -> hT_l0 tile (layer-0 output, stationary for L1)
        xc = None
        for t in range(T + LAG):
            if t < T:
                if t % CH == 0:
                    xc = xp.tile([128, CH * 128], F16, tag="xc")
                    nc.sync.dma_start(
                        xc[:].rearrange("p (t q) -> p t q", t=CH),
                        xT_d[:, t:t + CH, :])
                hT_l0 = lstm_step(0, t, hT_l0, xc, (t % CH) * 128, t == T - 1)
                ring[t] = hT_l0
            if t >= LAG and "no_l1" not in ABLATE:
                tt = t - LAG
                hT_l1 = lstm_step(1, tt, hT_l1, ring.pop(tt), 0, tt == T - 1)

    nc.compile()
    return nc


def _pack_inputs(x, Wi, Wh, bi, bh, direction):
    """Build per-core input map for one (direction, batch-half) role.
    x: [NB, T, IN] already batch-sliced (fp32). Wi/Wh: [L,4H,*], bi/bh: [L,4H]."""
    # gate reorder (i,f,g,o) -> (i,f,o,g)
    perm = np.concatenate([np.arange(0, H), np.arange(H, 2 * H),
                           np.arange(3 * H, 4 * H), np.arange(2 * H, 3 * H)])
    if direction:
        x = x[:, ::-1]
    # xT_pack[p, t, k*32+m] = x[m, t, k*128+p]
    xt = np.ascontiguousarray(x.transpose(2, 1, 0))          # [IN, T, NB]
    xt = xt.reshape(4, 128, T, NB).transpose(1, 2, 0, 3)     # [128, T, 4, NB]
    xT_pack = np.ascontiguousarray(xt.reshape(128, T, 4 * NB)).astype(np.float16)
    m = {"xT": xT_pack}
    for l in range(L):
        for nm, Wmat in [("WhT", Wh[l]), ("WiT", Wi[l])]:
            Wp = Wmat[perm]                                   # [4H, 512]
            WT = np.ascontiguousarray(Wp.T)                   # [512, 4H]
            WT = WT.reshape(4, 128, 4 * H).transpose(1, 0, 2) # [128, 4, 4H]
            m[f"{nm}{l}"] = np.ascontiguousarray(WT.reshape(128, 4 * 4 * H)).astype(np.float16)
        m[f"bias{l}"] = (bi[l] + bh[l])[perm].reshape(1, 4 * H).astype(np.float16)
    return m


def _make_in_maps(x, args):
    in_maps = []
    for core in range(4):
        d, half = core // 2, core % 2
        Wi, Wh, bi, bh = args[d]
        xs = x[half * NB:(half + 1) * NB]
        in_maps.append(_pack_inputs(xs, Wi, Wh, bi, bh, d))
    return in_maps + in_maps[:4]  # cores 4-7 duplicate


def kernel(x, W_ih_f, W_hh_f, b_ih_f, b_hh_f, W_ih_b, W_hh_b, b_ih_b, b_hh_b):
    global _compiled
    from concourse.bass_utils import run_bass_kernel_spmd

    x = np.asarray(x, dtype=np.float32)
    args = {
        0: (np.asarray(W_ih_f, np.float32), np.asarray(W_hh_f, np.float32),
            np.asarray(b_ih_f, np.float32), np.asarray(b_hh_f, np.float32)),
        1: (np.asarray(W_ih_b, np.float32), np.asarray(W_hh_b, np.float32),
            np.asarray(b_ih_b, np.float32), np.asarray(b_hh_b, np.float32)),
    }
    if _compiled is None:
        _compiled = _build_program()
    nc = _compiled
    in_maps = _make_in_maps(x, args)

    res = run_bass_kernel_spmd(nc, in_maps, list(range(N_CORES)))

    out = np.zeros((B, T, 2 * H), np.float32)
    h = np.zeros((L, B, 2 * H), np.float32)
    for core in range(4):
        d, half = core // 2, core % 2
        r = res.results[core]
        o = r["outs"]                      # [T, NB, H]
        if d:
            o = o[::-1]
        out[half * NB:(half + 1) * NB, :, d * H:(d + 1) * H] = o.transpose(1, 0, 2)
        h[:, half * NB:(half + 1) * NB, d * H:(d + 1) * H] = r["hfin"]
    c = h.copy()
    return out, h, c
